# revision 1
# baseline (speedup 1.0000x reference)
"""Trainium2 Bass kernel for nn_Attention4D_77644418777285.

Attention4D block (EfficientViT-style): 1x1-conv QKV + BN, depthwise-3x3
local-V branch, relative-position bias, talking-heads attention (8 heads,
49 tokens), projection. Batch 512 sharded 64-per-core across 8 NeuronCores
(pure data parallel; weights replicated).

v3 layout (per core, 64 images, groups of 8 images):
  - x arrives channel-major from host ([3,128,NT] bf16); output returned
    channel-major bf16 and re-transposed on host — no PE transposes.
  - Key-token axis (m) padded to 64 slots per image (8x8 grid, shared
    guard row/col, one-time zeroed): every shifted view is a plain 2-D
    column offset, v token-major tiles come from a single [128,128]
    PE transpose-matmul per (image-pair, channel-tile), and the attention
    middle (logits rows, talking-head blocks, selector, bias) runs in the
    64-slot row space with zero weights on guard slots.
  - depthwise 3x3 conv split between DVE (scalar_tensor_tensor taps into
    a B2-seeded SBUF accumulator) and PE (per-channel diagonal-matrix
    matmuls accumulating shifted windows in PSUM).
  - talking-heads + rel-pos bias injected pre-exp into the th1 psum chain;
    softmax denominators via selector matmul, batched DVE reciprocal,
    normalization broadcast via constant delta matmul.
  - assembly o + v_local(+psum part) + relu, projection, straight DMA out.
"""

import numpy as np
import ml_dtypes

R = 7
N = 49
H = 8
KD = 32
D = 128
DH = 1024
DIM = 384
SCALE = KD ** -0.5
NCORES = 8
B_FULL = 512
GW = 8 * N          # 392 compact cols per group of 8 images
SL = 64             # padded slots per image (8x8)
GWP = 8 * SL        # 512 padded cols per group
PADG = 16           # guard cols at each end of padded tiles

# conv tap split: (dy, dx) lists
TAPS_DVE = [(0, 0), (0, -1), (0, 1)]
TAPS_PE = [(-1, -1), (-1, 0), (-1, 1), (1, -1), (1, 0), (1, 1)]

_BF16 = ml_dtypes.bfloat16


def _bias_idxs(r):
    pos = np.stack(np.meshgrid(np.arange(r), np.arange(r))).reshape(2, -1)
    rel = np.abs(pos[:, :, None] - pos[:, None, :])
    return (rel[0] * r + rel[1]).reshape(-1)


def _slot(t):
    """compact token t (0..48) -> padded slot (0..63)."""
    return (t // 7) * 8 + (t % 7)


_SLOTS = np.array([_slot(t) for t in range(N)])


def make_consts(inp):
    f32 = np.float32
    g = {k: np.asarray(v, f32) for k, v in inp.items()}

    th1, th1_b = g['th1_w'], g['th1_b']
    th2, th2_b = g['th2_w'], g['th2_b']

    W_q = g['q_w'] * g['q_g'][None, :] * SCALE
    b_q = (g['q_b'] * g['q_g'] + g['q_beta']) * SCALE
    W_k = g['k_w'] * g['k_g'][None, :]
    b_k = g['k_b'] * g['k_g'] + g['k_beta']
    W_v = g['v_w'] * g['v_g'][None, :]
    b_v = g['v_b'] * g['v_g'] + g['v_beta']

    idxs = _bias_idxs(R)
    bias_full = g['attn_bias'][:, idxs].reshape(H, N, N)          # [h, n, m]
    biasp = np.einsum('hg,hnm->gnm', th1, bias_full) + th1_b[:, None, None]

    w9 = g['vl_w'].reshape(9, DH)                                  # [tap, c]
    w_eff = (w9 * g['vl_g'][None, :]).astype(f32)                  # [tap, c]
    sumw = np.zeros((DH, N), f32)
    for t in range(9):
        dy, dx = t // 3 - 1, t % 3 - 1
        for s in range(N):
            y, x = s // 7, s % 7
            if 0 <= y + dy < 7 and 0 <= x + dx < 7:
                sumw[:, s] += w9[t]
    s2 = th2.sum(axis=0) + N * th2_b                               # [g]
    B2 = (g['vl_g'][:, None] * (b_v[:, None] * sumw + g['vl_b'][:, None])
          + g['vl_beta'][:, None]
          + (b_v * s2[np.repeat(np.arange(H), D)])[:, None])       # [c, s=49]

    W_p = g['proj_w'] * g['proj_g'][None, :]
    b_p = g['proj_b'] * g['proj_g'] + g['proj_beta']

    consts = {}
    wqk = np.concatenate([W_q, W_k], axis=1).reshape(3, 128, 512)
    consts['wqk'] = wqk.astype(_BF16)
    consts['wv'] = W_v.reshape(3, 128, DH).astype(_BF16)
    consts['wp'] = W_p.reshape(8, 128, DIM).astype(_BF16)
    consts['bqk'] = np.concatenate([b_q, b_k]).reshape(4, 128).astype(f32)
    consts['bp'] = b_p.reshape(3, 128).astype(f32)

    # Talking heads as [jo, ji, K=128, M=128] block matrices in the
    # (hh, slot64) row space: row (hh*64 + slot(m)) of input tile ji =
    # head (2*ji+hh), key m; col likewise for output tile jo.
    def th_blocks(thw):
        Wb = np.zeros((4, 4, 128, 128), f32)
        eye = np.zeros((SL, SL), f32)
        eye[_SLOTS, _SLOTS] = 1.0
        for jo in range(4):
            for ji in range(4):
                for hhi in range(2):
                    for hho in range(2):
                        c = thw[2 * ji + hhi, 2 * jo + hho]
                        Wb[jo, ji, hhi * 64:hhi * 64 + SL,
                           hho * 64:hho * 64 + SL] += c * eye
        return Wb
    consts['w1s'] = th_blocks(th1).astype(_BF16)
    consts['w2s'] = th_blocks(th2).astype(_BF16)

    sel = np.zeros((128, 2), f32)
    sel[_SLOTS, 0] = 1.0
    sel[64 + _SLOTS, 1] = 1.0
    consts['sel'] = sel.astype(_BF16)

    dlt = np.zeros((128, 128), f32)
    for j in range(4):
        dlt[32 * j + 0, 0:64] = 1.0
        dlt[32 * j + 1, 64:128] = 1.0
    consts['dlt'] = dlt.astype(_BF16)

    # exp of the th1-transformed rel-pos bias in [(hh, slot) x (img, n)]
    # rows, replicated over 8 images (multiplied into E post-exp; guard
    # rows exp(0)=1).
    bsb = np.zeros((4, 128, GW), f32)
    for j in range(4):
        for hh in range(2):
            b = biasp[2 * j + hh].T                                # [m, n]
            bsb[j, hh * 64 + _SLOTS] = np.tile(b, (1, 8))
    consts['biasp'] = np.exp(bsb).astype(_BF16)

    # DVE tap weights: sbuf [128, 8, 9] (c-part, ct, tap)
    consts['w9t'] = w_eff.reshape(9, 8, 128).transpose(2, 1, 0).copy().astype(f32)

    # PE tap diagonal weights: [8 ct, n_pe, 128, 128]
    dw = np.zeros((8, len(TAPS_PE), 128, 128), f32)
    for ct in range(8):
        for ti, (dy, dx) in enumerate(TAPS_PE):
            tap = (dy + 1) * 3 + (dx + 1)
            np.fill_diagonal(dw[ct, ti], w_eff[tap, ct * 128:(ct + 1) * 128])
    consts['dw'] = dw.astype(_BF16)

    # B2 in padded-slot layout, replicated over the 8 images of a group:
    # [8, 128, 512] (zeros at guard slots)
    b2p = np.zeros((8, 128, SL), f32)
    b2p[:, :, _SLOTS] = B2.reshape(8, 128, N)
    consts['b2p'] = np.tile(b2p, (1, 1, 8)).astype(_BF16)

    consts['ident'] = np.eye(128, dtype=f32).astype(_BF16)
    return consts


def build_program(n_imgs, loop_n=1, stage=9):
    """Build the Bass program for one core processing n_imgs images.

    loop_n > 1 wraps the whole compute (including I/O DMA) in a hardware
    loop — used only by the timing harness to measure per-iteration HW time.
    """
    from contextlib import ExitStack
    import concourse.bass as bass
    import concourse.tile as tile
    from concourse import bacc, mybir

    f32 = mybir.dt.float32
    bf16 = mybir.dt.bfloat16
    AF = mybir.ActivationFunctionType
    ALU = mybir.AluOpType

    NI = n_imgs
    NG = NI // 8                 # groups of 8 images
    NT = NI * N                  # tokens

    nc = bacc.Bacc("TRN2", target_bir_lowering=False, debug=False,
                   enable_asserts=False)

    x_d = nc.dram_tensor("x", [3, 128, NT], bf16, kind="ExternalInput").ap()
    wqk_d = nc.dram_tensor("wqk", [3, 128, 512], bf16, kind="ExternalInput").ap()
    wv_d = nc.dram_tensor("wv", [3, 128, DH], bf16, kind="ExternalInput").ap()
    wp_d = nc.dram_tensor("wp", [8, 128, DIM], bf16, kind="ExternalInput").ap()
    bqk_d = nc.dram_tensor("bqk", [4, 128], f32, kind="ExternalInput").ap()
    bp_d = nc.dram_tensor("bp", [3, 128], f32, kind="ExternalInput").ap()
    w1_d = nc.dram_tensor("w1s", [4, 4, 128, 128], bf16, kind="ExternalInput").ap()
    w2_d = nc.dram_tensor("w2s", [4, 4, 128, 128], bf16, kind="ExternalInput").ap()
    sel_d = nc.dram_tensor("sel", [128, 2], bf16, kind="ExternalInput").ap()
    dlt_d = nc.dram_tensor("dlt", [128, 128], bf16, kind="ExternalInput").ap()
    bias_d = nc.dram_tensor("biasp", [4, 128, GW], bf16, kind="ExternalInput").ap()
    w9_d = nc.dram_tensor("w9t", [128, 8, 9], f32, kind="ExternalInput").ap()
    dw_d = nc.dram_tensor("dw", [8, len(TAPS_PE), 128, 128], bf16,
                          kind="ExternalInput").ap()
    b2_d = nc.dram_tensor("b2p", [8, 128, GWP], bf16, kind="ExternalInput").ap()
    id_d = nc.dram_tensor("ident", [128, 128], bf16, kind="ExternalInput").ap()
    out_d = nc.dram_tensor("out", [3, 128, NT], bf16, kind="ExternalOutput").ap()

    with tile.TileContext(nc) as tc, ExitStack() as ctx:
        const = ctx.enter_context(tc.tile_pool(name="const", bufs=1))
        pers = ctx.enter_context(tc.tile_pool(name="pers", bufs=1))
        mid = ctx.enter_context(tc.tile_pool(name="mid", bufs=6))
        accp = ctx.enter_context(tc.tile_pool(name="accp", bufs=2))
        stg = ctx.enter_context(tc.tile_pool(name="stg", bufs=3))
        ps = ctx.enter_context(tc.tile_pool(name="ps", bufs=6, space="PSUM"))
        psq = ctx.enter_context(tc.tile_pool(name="psq", bufs=2, space="PSUM"))

        dma = nc.sync.dma_start

        # ---------------- constants ----------------
        # issue order = scheduler priority: small / first-needed tiles first,
        # the large talking-heads + conv-diag tables last
        bqk_t = const.tile([128, 4], f32, name="bqk", tag="bqk")
        dma(out=bqk_t, in_=bass.AP(tensor=bqk_d.tensor, offset=0,
                                   ap=[[1, 128], [128, 4]]))
        bp_t = const.tile([128, 3], f32, name="bp", tag="bp")
        dma(out=bp_t, in_=bass.AP(tensor=bp_d.tensor, offset=0,
                                  ap=[[1, 128], [128, 3]]))
        sel_t = const.tile([128, 2], bf16, name="sel", tag="sel")
        dma(out=sel_t, in_=sel_d)
        dlt_t = const.tile([128, 128], bf16, name="dlt", tag="dlt")
        dma(out=dlt_t, in_=dlt_d)
        w9_t = const.tile([128, 8, 9], f32, name="w9", tag="w9")
        dma(out=w9_t, in_=w9_d)
        id_t = const.tile([128, 128], bf16, name="id", tag="id")
        dma(out=id_t, in_=id_d)
        wqk_t = [const.tile([128, 512], bf16, name=f"wqk{k}", tag=f"wqk{k}") for k in range(3)]
        wv_t = [const.tile([128, DH], bf16, name=f"wv{k}", tag=f"wv{k}") for k in range(3)]
        wp_t = [const.tile([128, DIM], bf16, name=f"wp{k}", tag=f"wp{k}") for k in range(8)]
        for k in range(3):
            dma(out=wqk_t[k], in_=wqk_d[k])
            dma(out=wv_t[k], in_=wv_d[k])
        bias_t = [const.tile([128, GW], bf16, name=f"bi{j}", tag=f"bi{j}") for j in range(4)]
        for j in range(4):
            dma(out=bias_t[j], in_=bias_d[j])
        b2c = [const.tile([128, GWP], bf16, name=f"b2c{c}", tag=f"b2c{c}")
               for c in range(8)]
        for c in range(8):
            dma(out=b2c[c], in_=b2_d[c])
        for k in range(8):
            dma(out=wp_t[k], in_=wp_d[k])
        w1_t = const.tile([128, 16, 128], bf16, name="w1", tag="w1")
        dma(out=w1_t, in_=bass.AP(tensor=w1_d.tensor, offset=0,
                                  ap=[[128, 128], [128 * 128, 16], [1, 128]]))
        w2_t = const.tile([128, 16, 128], bf16, name="w2", tag="w2")
        dma(out=w2_t, in_=bass.AP(tensor=w2_d.tensor, offset=0,
                                  ap=[[128, 128], [128 * 128, 16], [1, 128]]))
        dw_t = const.tile([128, 8 * len(TAPS_PE), 128], bf16, name="dw", tag="dw")
        dma(out=dw_t, in_=bass.AP(tensor=dw_d.tensor, offset=0,
                                  ap=[[128, 128], [128 * 128, 8 * len(TAPS_PE)],
                                      [1, 128]]))

        # ---------------- persistent / slot tiles ----------------
        NS = 3
        xg = [[pers.tile([128, GW], bf16, name=f"xg{k}_{s}", tag=f"xg{k}_{s}")
               for s in range(NS)] for k in range(3)]
        qcm = [[pers.tile([128, GW], bf16, name=f"q{t}_{s}", tag=f"q{t}_{s}")
                for s in range(NS)] for t in range(2)]
        kcm = [[pers.tile([128, GWP], bf16, name=f"k{t}_{s}", tag=f"k{t}_{s}")
                for s in range(NS)] for t in range(2)]
        vcm = [[pers.tile([128, GWP + 2 * PADG], bf16, name=f"vc{c}_{s}",
                          tag=f"vc{c}_{s}")
                for s in range(NS)] for c in range(8)]
        vtokE = [pers.tile([64, DH], bf16, name=f"vtE{s}", tag=f"vtE{s}") for s in range(8)]
        vtokO = [pers.tile([64, DH], bf16, name=f"vtO{s}", tag=f"vtO{s}") for s in range(8)]
        Ls = [[pers.tile([128, GW], bf16, name=f"Ls{j}_{s}", tag=f"Ls{j}_{s}") for s in range(2)]
              for j in range(4)]
        a2h = [[[pers.tile([64, GW], bf16, name=f"a2_{j}_{hh}_{s}",
                            tag=f"a2_{j}_{hh}_{s}") for s in range(2)]
                 for hh in range(2)] for j in range(4)]
        r_sb = [pers.tile([128, GW], bf16, name=f"rsb{s}", tag=f"rsb{s}") for s in range(2)]

        # one-time zero init: padded tiles fully (guard slots must stay 0)
        for c in range(8):
            for s in range(NS):
                nc.gpsimd.memset(vcm[c][s], 0.0)
        for t in range(2):
            for s in range(NS):
                nc.gpsimd.memset(kcm[t][s], 0.0)

        # 4-d views of a padded group region: [p, i, y(7), x(7)] valid slots
        def padview(tile_, base):
            v = tile_[:, base:base + GWP].rearrange("p (i q) -> p i q", q=SL)
            v = v.rearrange("p i (y x) -> p i y x", x=8)
            return v[:, :, 0:7, 0:7]

        def cview(tile_):
            return tile_.rearrange("p (i y x) -> p i y x", y=7, x=7)

        def group_body(g):
            sl = g % 2          # phase slot (middle tiles)
            s3 = g % NS         # deeper rotation for early tiles
            c0 = g * GW

            # --- x load (channel-major direct) ---
            if g == 0:
                with tc.high_priority():
                    for kt in range(3):
                        dma(out=xg[kt][s3], in_=x_d[kt][:, c0:c0 + GW])
            else:
                for kt in range(3):
                    dma(out=xg[kt][s3], in_=x_d[kt][:, c0:c0 + GW])

            # --- QKV channel-major ---
            for mt in range(12):
                qp = psq.tile([128, 512], f32, name="psq", tag="psq")
                for kt in range(3):
                    if mt < 4:
                        w = wqk_t[kt][:, mt * 128:(mt + 1) * 128]
                    else:
                        w = wv_t[kt][:, (mt - 4) * 128:(mt - 3) * 128]
                    nc.tensor.matmul(qp[:, 0:GW], w,
                                     xg[kt][s3],
                                     start=(kt == 0), stop=(kt == 2))
                if mt < 2:
                    nc.scalar.activation(qcm[mt][s3], qp[:, 0:GW],
                                         AF.Identity,
                                         bias=bqk_t[:, mt:mt + 1])
                elif mt < 4:
                    # k with bias, scattered into the padded-slot layout
                    nc.scalar.activation(padview(kcm[mt - 2][s3], 0),
                                         cview(qp[:, 0:GW]), AF.Identity,
                                         bias=bqk_t[:, mt:mt + 1])
                elif mt % 2 == 0:
                    nc.vector.tensor_copy(padview(vcm[mt - 4][s3], PADG),
                                          cview(qp[:, 0:GW]))
                else:
                    nc.scalar.activation(padview(vcm[mt - 4][s3], PADG),
                                         cview(qp[:, 0:GW]), AF.Copy)

            # --- v token-major via PE pair-transposes ---
            if stage < 2:
                return
            for pr in range(4):
                p = 4 * g + pr
                vp = [ps.tile([128, 512], f32, name="ps", tag="ps") for _ in range(2)]
                for ct in range(8):
                    nh, cc = ct // 4, (ct % 4) * 128
                    nc.tensor.matmul(
                        vp[nh][:, cc:cc + 128],
                        vcm[ct][s3][:, PADG + pr * 128: PADG + (pr + 1) * 128],
                        id_t, start=True, stop=True)
                nc.vector.tensor_copy(vtokE[p % 8][:, 0:512], vp[0][0:64, :])
                nc.scalar.activation(vtokO[p % 8][:, 0:512],
                                     vp[0][64:128, :], AF.Copy)
                nc.vector.tensor_copy(vtokE[p % 8][:, 512:1024], vp[1][0:64, :])
                nc.scalar.activation(vtokO[p % 8][:, 512:1024],
                                     vp[1][64:128, :], AF.Copy)

            # --- depthwise conv: DVE taps into B2-seeded acc ---
            if stage < 3:
                return
            acc_t = []
            for ct in range(8):
                acc = accp.tile([128, GWP], bf16, name=f"acc{ct}", tag=f"acc{ct}")
                accv3 = acc.rearrange("p (k x) -> p k x", x=8)
                b2v3 = b2c[ct].rearrange("p (k x) -> p k x", x=8)
                srcb = vcm[ct][s3]
                for dy, dx in TAPS_DVE:
                    tap = (dy + 1) * 3 + (dx + 1)
                    dlta = 8 * dy + dx
                    xs_o = slice(max(0, -dx), 7 - max(0, dx))
                    xs_i = slice(max(0, dx) + PADG % 8, 7 - max(0, -dx) + PADG % 8)
                    src3 = bass.AP(tensor=srcb.tensor,
                                   offset=srcb.offset + PADG + 8 * dy,
                                   ap=[[srcb.ap[0][0], 128], [8, 64], [1, 8]])
                    first = (dy, dx) == TAPS_DVE[0]
                    if first:
                        # seed guard slots too (assembly never reads them,
                        # but keep the accumulate chain well-defined)
                        nc.vector.scalar_tensor_tensor(
                            out=acc, in0=srcb[:, PADG:PADG + GWP],
                            scalar=w9_t[:, ct, tap:tap + 1],
                            in1=b2c[ct], op0=ALU.mult, op1=ALU.add)
                    else:
                        nc.vector.scalar_tensor_tensor(
                            out=accv3[:, :, xs_o],
                            in0=src3[:, :, xs_o.start + dx: xs_o.stop + dx],
                            scalar=w9_t[:, ct, tap:tap + 1],
                            in1=accv3[:, :, xs_o], op0=ALU.mult, op1=ALU.add)
                acc_t.append(acc)

            # --- qk logits (rows = (hh, slot64)) ---
            if stage < 4:
                return
            Lp = [ps.tile([128, 512], f32, name="ps", tag="ps") for _ in range(4)]
            for ig in range(8):
                for h in range(H):
                    j, hh = h // 2, h % 2
                    t4, row = h // 4, (h % 4) * 32
                    nc.tensor.matmul(
                        Lp[j][64 * hh: 64 * hh + SL, ig * N:(ig + 1) * N],
                        kcm[t4][s3][row:row + 32, ig * SL:(ig + 1) * SL],
                        qcm[t4][s3][row:row + 32, ig * N:(ig + 1) * N],
                        start=True, stop=True,
                        tile_position=(row, 64 * hh))
            for j in range(4):
                with tc.high_priority(700):
                    nc.scalar.activation(Ls[j][sl], Lp[j][:, 0:GW], AF.Copy)

            # --- talking heads 1 (+ rel-pos bias) + exp ---
            if stage < 5:
                return
            E = []
            L2p = [ps.tile([128, 512], f32, name="ps", tag="ps") for _ in range(4)]
            for jo in range(4):
                for ji in range(4):
                    nc.tensor.matmul(L2p[jo][:, 0:GW],
                                     w1_t[:, jo * 4 + ji, :],
                                     Ls[ji][sl],
                                     start=(ji == 0), stop=(ji == 3))
            for jo in range(4):
                e0 = mid.tile([128, GW], bf16, name="E0", tag="E0", bufs=4)
                e = mid.tile([128, GW], bf16, name="E", tag="E", bufs=8)
                with tc.high_priority(700):
                    nc.scalar.activation(e0, L2p[jo][:, 0:GW], AF.Exp)
                    nc.vector.tensor_mul(e, e0, bias_t[jo])
                E.append(e)

            # --- softmax denominator ---
            if stage < 6:
                return
            csp = ps.tile([128, 512], f32, name="ps", tag="ps")
            for j in range(4):
                nc.tensor.matmul(csp[32 * j: 32 * j + 2, 0:GW], sel_t, E[j],
                                 start=True, stop=True,
                                 tile_position=(0, 32 * j))
            with tc.high_priority(700):
                with nc.allow_low_precision(
                        reason="softmax denominators at bf16; 0.4% "
                               "normalization wobble vs 2e-2 tolerance"):
                    for j in range(4):
                        nc.vector.reciprocal(r_sb[sl][32 * j: 32 * j + 2, :],
                                             csp[32 * j: 32 * j + 2, 0:GW])

            # --- normalize + talking heads 2 ---
            A = []
            for j in range(4):
                rp = ps.tile([128, 512], f32, name="ps", tag="ps")
                nc.tensor.matmul(rp[:, 0:GW], dlt_t[32 * j: 32 * j + 2, :],
                                 r_sb[sl][32 * j: 32 * j + 2, :],
                                 start=True, stop=True,
                                 tile_position=(32 * j, 0))
                a = mid.tile([128, GW], bf16, name="A", tag="A", bufs=8)
                with tc.high_priority(700):
                    nc.vector.tensor_mul(a, E[j], rp[:, 0:GW])
                A.append(a)
            A2p = [ps.tile([128, 512], f32, name="ps", tag="ps") for _ in range(4)]
            for jo in range(4):
                for ji in range(4):
                    nc.tensor.matmul(A2p[jo][:, 0:GW],
                                     w2_t[:, jo * 4 + ji, :],
                                     A[ji],
                                     start=(ji == 0), stop=(ji == 3))
            for jo in range(4):
                with tc.high_priority(700):
                    nc.scalar.activation(a2h[jo][0][sl], A2p[jo][0:64, 0:GW],
                                         AF.Copy)
                    nc.scalar.activation(a2h[jo][1][sl], A2p[jo][64:128, 0:GW],
                                         AF.Copy)

            # --- attention * V (+ PE conv taps), assembly, relu ---
            if stage < 7:
                return
            relu_t = []
            for ct in range(8):
                op2 = ps.tile([128, 512], f32, name="ps", tag="ps")
                jo, hh = ct // 2, ct % 2
                for ig in range(8):
                    i = 8 * g + ig
                    pp = ig % 2
                    vt = (vtokE if pp == 0 else vtokO)[(i // 2) % 8]
                    nc.tensor.matmul(
                        op2[:, ig * N:(ig + 1) * N],
                        vt[0:SL, ct * 128:(ct + 1) * 128],
                        a2h[jo][hh][sl][0:SL, ig * N:(ig + 1) * N],
                        start=True, stop=True)
                if stage >= 8:
                    cps = ps.tile([128, 512], f32, name="ps", tag="ps")
                    for ti, (dy, dx) in enumerate(TAPS_PE):
                        dlta = 8 * dy + dx
                        nc.tensor.matmul(
                            cps[:, 0:GWP],
                            dw_t[:, ct * len(TAPS_PE) + ti, :],
                            vcm[ct][s3][:, PADG + dlta: PADG + dlta + GWP],
                            start=(ti == 0), stop=(ti == len(TAPS_PE) - 1))
                tmp = mid.tile([128, GW], bf16, name="tmp", tag="tmp", bufs=5)
                op2d = mid.tile([128, GW], bf16, name="op2d", tag="op2d", bufs=5)
                opv = op2[:, 0:GW].rearrange("p (i y x) -> p i y x", y=7, x=7)
                nc.scalar.activation(op2d, op2[:, 0:GW], AF.Copy)
                accv = acc_t[ct].rearrange("p (i q) -> p i q", q=SL)
                accv = accv.rearrange("p i (y x) -> p i y x",
                                      x=8)[:, :, 0:7, 0:7]
                nc.vector.tensor_add(cview(tmp), cview(op2d), accv)
                if stage >= 8:
                    cpsd = mid.tile([128, GW], bf16, name="cpsd", tag="cpsd",
                                    bufs=5)
                    cpsv = cps[:, 0:GWP].rearrange("p (i q) -> p i q", q=SL)
                    cpsv = cpsv.rearrange("p i (y x) -> p i y x", x=8)[:, :, 0:7, 0:7]
                    nc.scalar.activation(cview(cpsd), cpsv, AF.Copy)
                    nc.vector.tensor_add(cview(tmp), cview(tmp), cview(cpsd))
                rl = mid.tile([128, GW], bf16, name="rl", tag="rl", bufs=10)
                nc.vector.tensor_scalar_max(rl, tmp, 0.0)
                relu_t.append(rl)

            # --- projection + store ---
            for mt in range(3):
                st = stg.tile([128, GW], bf16, name="st", tag="st")
                pp_ = ps.tile([128, 512], f32, name="ps", tag="ps")
                for kt in range(8):
                    nc.tensor.matmul(pp_[:, 0:GW],
                                     wp_t[kt][:, mt * 128:(mt + 1) * 128],
                                     relu_t[kt],
                                     start=(kt == 0), stop=(kt == 7))
                nc.scalar.activation(st, pp_[:, 0:GW], AF.Identity,
                                     bias=bp_t[:, mt:mt + 1])
                dma(out=out_d[mt][:, c0:c0 + GW], in_=st)

        if loop_n > 1:
            with tc.For_i(0, loop_n, 1):
                for g in range(NG):
                    group_body(g)
        else:
            with tc.high_priority(200):
                group_body(0)
            for g in range(1, NG):
                group_body(g)

    nc.compile()
    return nc


_CACHE = {}


def _get_program(n_imgs):
    if n_imgs not in _CACHE:
        _CACHE[n_imgs] = build_program(n_imgs)
    return _CACHE[n_imgs]


_CONSTS_CACHE = {}


def _cached_consts(inputs):
    w = np.asarray(inputs['q_w'])
    key = (w.shape, w.dtype.str, w.tobytes()[:256])
    if key not in _CONSTS_CACHE:
        _CONSTS_CACHE.clear()
        _CONSTS_CACHE[key] = make_consts(inputs)
    return _CONSTS_CACHE[key]


def make_in_maps(inputs, n_cores=NCORES):
    """Host prep: shard + channel-major x, build replicated constants."""
    consts = _cached_consts(inputs)
    x = np.asarray(inputs['x'], np.float32)
    B = x.shape[0]
    ni = B // n_cores
    nt = ni * N
    x = x.reshape(B, N, DIM)
    in_maps = []
    for c in range(n_cores):
        m = dict(consts)
        xc = x[c * ni:(c + 1) * ni].reshape(nt, DIM).T    # [384, nt]
        m['x'] = np.ascontiguousarray(xc).reshape(3, 128, nt).astype(_BF16)
        in_maps.append(m)
    return in_maps, ni


def assemble_out(results, ni):
    """[3,128,nt] bf16 per core -> full [B, R, R, DIM] f32."""
    nt = ni * N
    outs = []
    for r in results:
        oc = np.asarray(r['out'], np.float32).reshape(DIM, nt)
        outs.append(oc.T.reshape(ni, R, R, DIM))
    return np.concatenate(outs, axis=0)


def kernel(**inputs):
    from concourse import bass_utils
    in_maps, ni = make_in_maps(inputs)
    nc = _get_program(ni)
    res = bass_utils.run_bass_kernel_spmd(
        nc, in_maps, core_ids=list(range(NCORES)))
    return assemble_out(res.results, ni).astype(np.float32)



# revision 50
# speedup vs baseline: 1.2244x; 1.2244x over previous
"""Trainium2 Bass kernel for nn_Attention4D_77644418777285.

Attention4D block (EfficientViT-style): 1x1-conv QKV + BN, depthwise-3x3
local-V branch, relative-position bias, talking-heads attention (8 heads,
49 tokens), projection. Batch 512 sharded 64-per-core across 8 NeuronCores
(pure data parallel; weights replicated).

v4 layout (per core, 64 images, groups of 8 images):
  - x arrives channel-major from host ([3,128,NT] bf16); output returned
    channel-major bf16 (group images in IPERM order) and fixed up on host.
  - v carries its BN bias (v' = xWv + b_v, added at PSUM evacuation), so
    the zero guard slots match the reference SAME-padding, the attention
    bias correction is implicit in A2 @ v', and the local branch only needs
    a per-channel constant folded into the relu.
  - Key-token axis (m) padded to 64 slots per image (8x8 grid, shared
    guard row/col, one-time zeroed); the attention middle (logits rows,
    talking-head blocks, selector) runs in the 64-slot row space with zero
    weights on guard slots. Rel-pos bias (log domain, th1-transformed) is
    injected into the th1 psum chain via an identity matmul with a
    stride-0-broadcast rhs; one batched reciprocal covers all denominators.
  - vcm holds images in IPERM (pair-major) order via a 5-d permuted
    scatter; v token-major [128,1024] pair tiles come from PE transposes
    with full-height evacuations; A2 is rebuilt as checkerboard tiles
    (direct ACT copies for the aligned quadrants, SBUF-to-SBUF DMAs for the
    partition-crossing ones) so attn*V runs as K=128 image-pair matmuls
    accumulating into the same psum as the PE conv taps.
  - depthwise 3x3 conv: seed via 4x-mode tensor_scalar, row/column taps as
    DVE scalar_tensor_tensor, corners (+(-1,0) for ct<4) as per-channel
    diagonal matmuls with compact shifted rhs views into the attn psum.
    GPSIMD runs no tensor compute (10-30x slower on HW than modeled).
  - assembly: one DVE add (psum + acc) + ACT relu(+bias), projection,
    straight DMA out.
"""

import numpy as np
import ml_dtypes

R = 7
N = 49
H = 8
KD = 32
D = 128
DH = 1024
DIM = 384
SCALE = KD ** -0.5
NCORES = 8
B_FULL = 512
GW = 8 * N          # 392 compact cols per group of 8 images
SL = 64             # padded slots per image (8x8)
GWP = 8 * SL        # 512 padded cols per group
PADG = 16           # guard cols at each end of padded tiles

# conv tap split: seed/(1,0)/(0,+-1) on DVE, 4 corners on PE, and (-1,0)
# on PE for ct<4 / DVE for ct>=4 (engine balance). Pool runs no tensor
# compute (GPSIMD tensor ops are ~10-30x slower on HW than modeled).
TAPS_PE = [(-1, -1), (-1, 1), (1, -1), (1, 1), (-1, 0)]
N_PE_TAPS = {ct: 5 for ct in range(8)}

# vcm/output image order within a group: vcm slot v holds middle image
# IPERM[v]; pairs (2p, 2p+1) = middle images (p, p+4), so pair-firsts span
# compact cols 0:196 and pair-seconds 196:392 in the attention middle.
IPERM = [0, 4, 1, 5, 2, 6, 3, 7]
IPERM_INV = [0, 2, 4, 6, 1, 3, 5, 7]

_BF16 = ml_dtypes.bfloat16


def _bias_idxs(r):
    pos = np.stack(np.meshgrid(np.arange(r), np.arange(r))).reshape(2, -1)
    rel = np.abs(pos[:, :, None] - pos[:, None, :])
    return (rel[0] * r + rel[1]).reshape(-1)


def _slot(t):
    """compact token t (0..48) -> padded slot (0..63)."""
    return (t // 7) * 8 + (t % 7)


_SLOTS = np.array([_slot(t) for t in range(N)])


def make_consts(inp):
    f32 = np.float32
    g = {k: np.asarray(v, f32) for k, v in inp.items()}

    th1, th1_b = g['th1_w'], g['th1_b']
    th2, th2_b = g['th2_w'], g['th2_b']

    W_q = g['q_w'] * g['q_g'][None, :] * SCALE
    b_q = (g['q_b'] * g['q_g'] + g['q_beta']) * SCALE
    W_k = g['k_w'] * g['k_g'][None, :]
    b_k = g['k_b'] * g['k_g'] + g['k_beta']
    W_v = g['v_w'] * g['v_g'][None, :]
    b_v = g['v_b'] * g['v_g'] + g['v_beta']

    idxs = _bias_idxs(R)
    bias_full = g['attn_bias'][:, idxs].reshape(H, N, N)          # [h, n, m]
    biasp = np.einsum('hg,hnm->gnm', th1, bias_full) + th1_b[:, None, None]

    w9 = g['vl_w'].reshape(9, DH)                                  # [tap, c]
    w_eff = (w9 * g['vl_g'][None, :]).astype(f32)                  # [tap, c]
    W_p = g['proj_w'] * g['proj_g'][None, :]
    b_p = g['proj_b'] * g['proj_g'] + g['proj_beta']

    consts = {}
    wqk = np.concatenate([W_q, W_k], axis=1).reshape(3, 128, 512)
    consts['wqk'] = wqk.astype(_BF16)
    consts['wv'] = W_v.reshape(3, 128, DH).astype(_BF16)
    consts['wp'] = W_p.reshape(8, 128, DIM).astype(_BF16)
    # v carries its bias (v' = xWv + b_v) so the conv's zero guards match the
    # reference SAME-padding and the attention bias correction is automatic;
    # the remaining local-branch bias is the per-channel crelu constant.
    consts['bqv'] = np.concatenate([b_q, b_k, b_v]).reshape(12, 128).astype(f32)
    consts['crelu'] = (g['vl_g'] * g['vl_b']
                       + g['vl_beta']).reshape(8, 128).astype(f32)
    consts['bp'] = b_p.reshape(3, 128).astype(f32)

    # Talking heads as [jo, ji, K=128, M=128] block matrices in the
    # (hh, slot64) row space: row (hh*64 + slot(m)) of input tile ji =
    # head (2*ji+hh), key m; col likewise for output tile jo.
    def th_blocks(thw):
        Wb = np.zeros((4, 4, 128, 128), f32)
        eye = np.zeros((SL, SL), f32)
        eye[_SLOTS, _SLOTS] = 1.0
        for jo in range(4):
            for ji in range(4):
                for hhi in range(2):
                    for hho in range(2):
                        c = thw[2 * ji + hhi, 2 * jo + hho]
                        Wb[jo, ji, hhi * 64:hhi * 64 + SL,
                           hho * 64:hho * 64 + SL] += c * eye
        return Wb
    consts['w1s'] = th_blocks(th1).astype(_BF16)
    consts['w2s'] = th_blocks(th2).astype(_BF16)

    sel = np.zeros((128, 2), f32)
    sel[_SLOTS, 0] = 1.0
    sel[64 + _SLOTS, 1] = 1.0
    consts['sel'] = sel.astype(_BF16)

    dlt = np.zeros((128, 128), f32)
    for j in range(4):
        dlt[32 * j + 0, 0:64] = 1.0
        dlt[32 * j + 1, 64:128] = 1.0
    consts['dlt'] = dlt.astype(_BF16)

    # th1-transformed rel-pos bias (log domain) in [(hh, slot) x n] rows for
    # ONE image; injected into the L2 psum chain via an identity matmul with
    # a stride-0-broadcast rhs. Guard rows add 0.
    bsb = np.zeros((4, 128, N), f32)
    for j in range(4):
        for hh in range(2):
            b = biasp[2 * j + hh].T                                # [m, n]
            bsb[j, hh * 64 + _SLOTS] = b
    consts['biasp'] = bsb.astype(_BF16)

    # DVE tap weights: sbuf [128, 8, 9] (c-part, ct, tap)
    consts['w9t'] = w_eff.reshape(9, 8, 128).transpose(2, 1, 0).copy().astype(f32)

    # PE tap diagonal weights: [8 ct, n_pe, 128, 128]
    dw = np.zeros((8, len(TAPS_PE), 128, 128), f32)
    for ct in range(8):
        for ti, (dy, dx) in enumerate(TAPS_PE):
            tap = (dy + 1) * 3 + (dx + 1)
            np.fill_diagonal(dw[ct, ti], w_eff[tap, ct * 128:(ct + 1) * 128])
    consts['dw'] = dw.astype(_BF16)

    consts['ident'] = np.eye(128, dtype=f32).astype(_BF16)
    return consts


def build_program(n_imgs, loop_n=1, stage=9):
    """Build the Bass program for one core processing n_imgs images.

    loop_n > 1 wraps the whole compute (including I/O DMA) in a hardware
    loop — used only by the timing harness to measure per-iteration HW time.
    """
    from contextlib import ExitStack
    import concourse.bass as bass
    import concourse.tile as tile
    from concourse import bacc, mybir

    f32 = mybir.dt.float32
    bf16 = mybir.dt.bfloat16
    AF = mybir.ActivationFunctionType
    ALU = mybir.AluOpType

    NI = n_imgs
    NG = NI // 8                 # groups of 8 images
    NT = NI * N                  # tokens

    nc = bacc.Bacc("TRN2", target_bir_lowering=False, debug=False,
                   enable_asserts=False)

    x_d = nc.dram_tensor("x", [3, 128, NT], bf16, kind="ExternalInput").ap()
    wqk_d = nc.dram_tensor("wqk", [3, 128, 512], bf16, kind="ExternalInput").ap()
    wv_d = nc.dram_tensor("wv", [3, 128, DH], bf16, kind="ExternalInput").ap()
    wp_d = nc.dram_tensor("wp", [8, 128, DIM], bf16, kind="ExternalInput").ap()
    bqv_d = nc.dram_tensor("bqv", [12, 128], f32, kind="ExternalInput").ap()
    crl_d = nc.dram_tensor("crelu", [8, 128], f32, kind="ExternalInput").ap()
    bp_d = nc.dram_tensor("bp", [3, 128], f32, kind="ExternalInput").ap()
    w1_d = nc.dram_tensor("w1s", [4, 4, 128, 128], bf16, kind="ExternalInput").ap()
    w2_d = nc.dram_tensor("w2s", [4, 4, 128, 128], bf16, kind="ExternalInput").ap()
    sel_d = nc.dram_tensor("sel", [128, 2], bf16, kind="ExternalInput").ap()
    dlt_d = nc.dram_tensor("dlt", [128, 128], bf16, kind="ExternalInput").ap()
    bias_d = nc.dram_tensor("biasp", [4, 128, N], bf16, kind="ExternalInput").ap()
    w9_d = nc.dram_tensor("w9t", [128, 8, 9], f32, kind="ExternalInput").ap()
    dw_d = nc.dram_tensor("dw", [8, len(TAPS_PE), 128, 128], bf16,
                          kind="ExternalInput").ap()
    id_d = nc.dram_tensor("ident", [128, 128], bf16, kind="ExternalInput").ap()
    out_d = nc.dram_tensor("out", [3, 128, NT], bf16, kind="ExternalOutput").ap()

    with tile.TileContext(nc) as tc, ExitStack() as ctx:
        const = ctx.enter_context(tc.tile_pool(name="const", bufs=1))
        pers = ctx.enter_context(tc.tile_pool(name="pers", bufs=1))
        mid = ctx.enter_context(tc.tile_pool(name="mid", bufs=6))
        accp = ctx.enter_context(tc.tile_pool(name="accp", bufs=3))
        stg = ctx.enter_context(tc.tile_pool(name="stg", bufs=3))
        ps = ctx.enter_context(tc.tile_pool(name="ps", bufs=5, space="PSUM"))
        psq = ctx.enter_context(tc.tile_pool(name="psq", bufs=3, space="PSUM"))

        dma = nc.sync.dma_start

        # ---------------- persistent / slot tiles ----------------
        NS = 3
        xg = [[pers.tile([128, GW], bf16, name=f"xg{k}_{s}", tag=f"xg{k}_{s}")
               for s in range(NS)] for k in range(3)]
        qcm = [[pers.tile([128, GW], bf16, name=f"q{t}_{s}", tag=f"q{t}_{s}")
                for s in range(NS)] for t in range(2)]
        kcm = [[pers.tile([128, GWP], bf16, name=f"k{t}_{s}", tag=f"k{t}_{s}")
                for s in range(NS)] for t in range(2)]
        vcm = [[pers.tile([128, GWP + 2 * PADG], bf16, name=f"vc{c}_{s}",
                          tag=f"vc{c}_{s}")
                for s in range(NS)] for c in range(8)]
        vtp = [pers.tile([128, DH], bf16, name=f"vtp{s}", tag=f"vtp{s}")
               for s in range(8)]
        Ls = [[pers.tile([128, GW], bf16, name=f"Ls{j}_{s}", tag=f"Ls{j}_{s}") for s in range(3)]
              for j in range(4)]
        # checkerboard A2 tiles: rows 0:64 = head (jo,hh) values for
        # pair-first images (cols 0:196), rows 64:128 = pair-second images
        # (cols 196:392); the complementary quadrants stay zero forever.
        chk = [[[pers.tile([128, GW], bf16, name=f"ck{j}_{hh}_{s}",
                           tag=f"ck{j}_{hh}_{s}") for s in range(3)]
                for hh in range(2)] for j in range(4)]
        r_sb = [pers.tile([128, GW], bf16, name=f"rsb{s}", tag=f"rsb{s}") for s in range(3)]

        # ---------------- constants ----------------
        # issue order = scheduler priority: small / first-needed tiles first,
        # the large talking-heads + conv-diag tables last
        bqv_t = const.tile([128, 12], f32, name="bqv", tag="bqv")
        dma(out=bqv_t, in_=bass.AP(tensor=bqv_d.tensor, offset=0,
                                   ap=[[1, 128], [128, 12]]))
        crl_t = const.tile([128, 8], f32, name="crl", tag="crl")
        dma(out=crl_t, in_=bass.AP(tensor=crl_d.tensor, offset=0,
                                   ap=[[1, 128], [128, 8]]))
        bp_t = const.tile([128, 3], f32, name="bp", tag="bp")
        dma(out=bp_t, in_=bass.AP(tensor=bp_d.tensor, offset=0,
                                  ap=[[1, 128], [128, 3]]))
        sel_t = const.tile([128, 2], bf16, name="sel", tag="sel")
        dma(out=sel_t, in_=sel_d)
        dlt_t = const.tile([128, 128], bf16, name="dlt", tag="dlt")
        dma(out=dlt_t, in_=dlt_d)
        w9_t = const.tile([128, 8, 9], f32, name="w9", tag="w9")
        dma(out=w9_t, in_=w9_d)
        id_t = const.tile([128, 128], bf16, name="id", tag="id")
        dma(out=id_t, in_=id_d)
        # group-0 x preload ahead of the big constant tables (startup fill)
        if loop_n == 1:
            with tc.high_priority():
                for kt in range(3):
                    dma(out=xg[kt][0], in_=x_d[kt][:, 0:GW])
        wqk_t = [const.tile([128, 512], bf16, name=f"wqk{k}", tag=f"wqk{k}") for k in range(3)]
        wv_t = [const.tile([128, DH], bf16, name=f"wv{k}", tag=f"wv{k}") for k in range(3)]
        wp_t = [const.tile([128, DIM], bf16, name=f"wp{k}", tag=f"wp{k}") for k in range(8)]
        for k in range(3):
            dma(out=wqk_t[k], in_=wqk_d[k])
            dma(out=wv_t[k], in_=wv_d[k])
        bias_t = [const.tile([128, N], bf16, name=f"bi{j}", tag=f"bi{j}") for j in range(4)]
        for j in range(4):
            dma(out=bias_t[j], in_=bias_d[j])
        for k in range(8):
            dma(out=wp_t[k], in_=wp_d[k])
        w1_t = const.tile([128, 16, 128], bf16, name="w1", tag="w1")
        dma(out=w1_t, in_=bass.AP(tensor=w1_d.tensor, offset=0,
                                  ap=[[128, 128], [128 * 128, 16], [1, 128]]))
        w2_t = const.tile([128, 16, 128], bf16, name="w2", tag="w2")
        dma(out=w2_t, in_=bass.AP(tensor=w2_d.tensor, offset=0,
                                  ap=[[128, 128], [128 * 128, 16], [1, 128]]))
        dw_t = const.tile([128, 8 * len(TAPS_PE), 128], bf16, name="dw", tag="dw")
        dma(out=dw_t, in_=bass.AP(tensor=dw_d.tensor, offset=0,
                                  ap=[[128, 128], [128 * 128, 8 * len(TAPS_PE)],
                                      [1, 128]]))

        # one-time zero init: padded tiles fully (guard slots must stay 0),
        # spread across Pool/DVE/ACT so the wall time is ~1/3
        ms_tiles = ([vcm[c][s] for c in range(8) for s in range(NS)]
                    + [kcm[t][s] for t in range(2) for s in range(NS)]
                    + [chk[j][hh][s] for j in range(4) for hh in range(2)
                       for s in range(3)])
        for mi, tile_ in enumerate(ms_tiles):
            if mi % 2 == 0:
                nc.gpsimd.memset(tile_, 0.0)
            else:
                nc.scalar.memzero(tile_)

        # 4-d views of a padded group region: [p, i, y(7), x(7)] valid slots
        def padview(tile_, base):
            v = tile_[:, base:base + GWP].rearrange("p (i q) -> p i q", q=SL)
            v = v.rearrange("p i (y x) -> p i y x", x=8)
            return v[:, :, 0:7, 0:7]

        def cview(tile_):
            return tile_.rearrange("p (i y x) -> p i y x", y=7, x=7)

        def group_body(g):
            sl = g % 3          # phase slot (middle tiles)
            s3 = g % NS         # deeper rotation for early tiles
            c0 = g * GW

            # --- x load (channel-major direct; g=0 preloaded up top) ---
            if g > 0 or loop_n > 1:
                for kt in range(3):
                    dma(out=xg[kt][s3], in_=x_d[kt][:, c0:c0 + GW])

            # --- QKV channel-major ---
            for mt in range(12):
                qp = psq.tile([128, 512], f32, name="psq", tag="psq")
                for kt in range(3):
                    if mt < 4:
                        w = wqk_t[kt][:, mt * 128:(mt + 1) * 128]
                    else:
                        w = wv_t[kt][:, (mt - 4) * 128:(mt - 3) * 128]
                    nc.tensor.matmul(qp[:, 0:GW], w,
                                     xg[kt][s3],
                                     start=(kt == 0), stop=(kt == 2))
                if mt < 2:
                    nc.scalar.activation(qcm[mt][s3], qp[:, 0:GW],
                                         AF.Identity,
                                         bias=bqv_t[:, mt:mt + 1])
                elif mt < 4:
                    # k with bias, scattered into the padded-slot layout
                    nc.scalar.activation(padview(kcm[mt - 2][s3], 0),
                                         cview(qp[:, 0:GW]), AF.Identity,
                                         bias=bqv_t[:, mt:mt + 1])
                else:
                    # v scattered into padded slots, image-permuted so vcm
                    # slot 2a+b holds middle image a+4b (5-d affine scatter)
                    vt_ = vcm[mt - 4][s3]
                    dst = bass.AP(tensor=vt_.tensor,
                                  offset=vt_.offset + PADG,
                                  ap=[[vt_.ap[0][0], 128], [128, 4], [64, 2],
                                      [8, 7], [1, 7]])
                    src = bass.AP(tensor=qp.tensor, offset=qp.offset,
                                  ap=[[qp.ap[0][0], 128], [49, 4], [196, 2],
                                      [7, 7], [1, 7]])
                    nc.scalar.activation(dst, src, AF.Identity,
                                         bias=bqv_t[:, mt:mt + 1])

            # --- v token-major via PE pair-transposes ---
            if stage < 2:
                return
            for pr in range(4):
                p = 4 * g + pr
                vp = [ps.tile([128, 512], f32, name="ps", tag="ps") for _ in range(2)]
                for ct in range(8):
                    nh, cc = ct // 4, (ct % 4) * 128
                    nc.tensor.matmul(
                        vp[nh][:, cc:cc + 128],
                        vcm[ct][s3][:, PADG + pr * 128: PADG + (pr + 1) * 128],
                        id_t, start=True, stop=True)
                nc.vector.tensor_copy(vtp[p % 8][:, 0:512], vp[0])
                nc.scalar.activation(vtp[p % 8][:, 512:1024], vp[1], AF.Copy)

            # --- depthwise conv row taps: DVE chain + Pool hybrid products ---
            if stage < 3:
                return
            acc_t = []
            for ct in range(8):
                acc = accp.tile([128, GWP], bf16, name=f"acc{ct}", tag=f"acc{ct}")
                accv3 = acc.rearrange("p (k x) -> p k x", x=8)
                srcb = vcm[ct][s3]
                # seed: acc = w(0,0) * v  (single-src, 4x mode)
                nc.vector.tensor_scalar_mul(acc, srcb[:, PADG:PADG + GWP],
                                            w9_t[:, ct, 4:5])
                # (1,0): contiguous in-place stt
                for dy in (1,):
                    tap = (dy + 1) * 3 + 1
                    nc.vector.scalar_tensor_tensor(
                        out=acc,
                        in0=srcb[:, PADG + 8 * dy:PADG + 8 * dy + GWP],
                        scalar=w9_t[:, ct, tap:tap + 1], in1=acc,
                        op0=ALU.mult, op1=ALU.add)
                # (0,-1) / (0,1): x-sliced in-place stt
                src3 = bass.AP(tensor=srcb.tensor,
                               offset=srcb.offset + PADG,
                               ap=[[srcb.ap[0][0], 128], [8, 64], [1, 8]])
                nc.vector.scalar_tensor_tensor(
                    out=accv3[:, :, 1:7], in0=src3[:, :, 0:6],
                    scalar=w9_t[:, ct, 3:4], in1=accv3[:, :, 1:7],
                    op0=ALU.mult, op1=ALU.add)
                nc.vector.scalar_tensor_tensor(
                    out=accv3[:, :, 0:7], in0=src3[:, :, 1:8],
                    scalar=w9_t[:, ct, 5:6], in1=accv3[:, :, 0:7],
                    op0=ALU.mult, op1=ALU.add)
                acc_t.append(acc)

            # --- qk logits (rows = (hh, slot64)) ---
            if stage < 4:
                return
            Lp = [ps.tile([128, 512], f32, name="ps", tag="ps") for _ in range(4)]
            for ig in range(8):
                for h in range(H):
                    j, hh = h // 2, h % 2
                    t4, row = h // 4, (h % 4) * 32
                    nc.tensor.matmul(
                        Lp[j][64 * hh: 64 * hh + SL, ig * N:(ig + 1) * N],
                        kcm[t4][s3][row:row + 32, ig * SL:(ig + 1) * SL],
                        qcm[t4][s3][row:row + 32, ig * N:(ig + 1) * N],
                        start=True, stop=True,
                        tile_position=(row, 64 * hh))
            for j in range(4):
                with tc.high_priority(700):
                    nc.scalar.activation(Ls[j][sl], Lp[j][:, 0:GW], AF.Copy)

            # --- talking heads 1 (+ rel-pos bias) + exp ---
            if stage < 5:
                return
            E = []
            L2p = [ps.tile([128, 512], f32, name="ps", tag="ps") for _ in range(4)]
            for jo in range(4):
                # rel-pos bias (log domain) first, one image broadcast over 8
                bib = bass.AP(tensor=bias_t[jo].tensor,
                              offset=bias_t[jo].offset,
                              ap=[[bias_t[jo].ap[0][0], 128], [0, 8], [1, N]])
                nc.tensor.matmul(L2p[jo][:, 0:GW], id_t, bib,
                                 start=True, stop=False)
                for ji in range(4):
                    nc.tensor.matmul(L2p[jo][:, 0:GW],
                                     w1_t[:, jo * 4 + ji, :],
                                     Ls[ji][sl],
                                     start=False, stop=(ji == 3))
            for jo in range(4):
                e = mid.tile([128, GW], bf16, name="E", tag="E", bufs=8)
                with tc.high_priority(700):
                    nc.scalar.activation(e, L2p[jo][:, 0:GW], AF.Exp)
                E.append(e)

            # --- softmax denominator ---
            if stage < 6:
                return
            csp = ps.tile([128, 512], f32, name="ps", tag="ps")
            for j in range(4):
                nc.tensor.matmul(csp[32 * j: 32 * j + 2, 0:GW], sel_t, E[j],
                                 start=True, stop=True,
                                 tile_position=(0, 32 * j))
            with tc.high_priority(700):
                with nc.allow_low_precision(
                        reason="softmax denominators at bf16; 0.4% "
                               "normalization wobble vs 2e-2 tolerance"):
                    # one op covers all 4 denominator row-pairs (rows 32j..
                    # 32j+1); the in-between rows hold stale psum, their
                    # reciprocals land in unread r_sb rows
                    nc.vector.reciprocal(r_sb[sl][0:98, :], csp[0:98, 0:GW])

            # --- normalize + talking heads 2 ---
            A = []
            for j in range(4):
                rp = ps.tile([128, 512], f32, name="ps", tag="ps")
                nc.tensor.matmul(rp[:, 0:GW], dlt_t[32 * j: 32 * j + 2, :],
                                 r_sb[sl][32 * j: 32 * j + 2, :],
                                 start=True, stop=True,
                                 tile_position=(32 * j, 0))
                a = mid.tile([128, GW], bf16, name="A", tag="A", bufs=8)
                with tc.high_priority(700):
                    nc.vector.tensor_mul(a, E[j], rp[:, 0:GW])
                A.append(a)
            A2p = [ps.tile([128, 512], f32, name="ps", tag="ps") for _ in range(4)]
            for jo in range(4):
                for ji in range(4):
                    nc.tensor.matmul(A2p[jo][:, 0:GW],
                                     w2_t[:, jo * 4 + ji, :],
                                     A[ji],
                                     start=(ji == 0), stop=(ji == 3))
            HW = 4 * N          # 196: pair-first half of the compact cols
            for jo in range(4):
                tx = mid.tile([128, HW], bf16, name="tx", tag="tx", bufs=4)
                with tc.high_priority(700):
                    # direct quadrants (partition-aligned)
                    nc.scalar.activation(chk[jo][0][sl][0:64, 0:HW],
                                         A2p[jo][0:64, 0:HW], AF.Copy)
                    nc.scalar.activation(chk[jo][1][sl][64:128, HW:2 * HW],
                                         A2p[jo][64:128, HW:2 * HW], AF.Copy)
                    # cross quadrants: stage at source partitions, then a
                    # contiguous SBUF->SBUF DMA moves them across partitions
                    nc.scalar.activation(tx[0:64, :],
                                         A2p[jo][0:64, HW:2 * HW], AF.Copy)
                    nc.scalar.activation(tx[64:128, :],
                                         A2p[jo][64:128, 0:HW], AF.Copy)
                dma(out=chk[jo][0][sl][64:128, HW:2 * HW], in_=tx[0:64, :])
                dma(out=chk[jo][1][sl][0:64, 0:HW], in_=tx[64:128, :])

            # --- attention * V + PE conv taps into one psum, combine, relu ---
            if stage < 7:
                return
            relu_t = []
            for ct in range(8):
                op2 = ps.tile([128, 512], f32, name="ps", tag="ps")
                jo, hh = ct // 2, ct % 2
                srcb = vcm[ct][s3]
                if stage >= 8:
                    # depthwise taps: compact strided rhs views of the padded
                    # source, accumulating straight into the attn-out psum
                    for ti in range(N_PE_TAPS[ct]):
                        dy, dx = TAPS_PE[ti]
                        if ti == 0:
                            ys, xs = slice(0, 7), slice(0, 7)
                        else:
                            ys = slice(max(0, -dy), 7 - max(0, dy))
                            xs = slice(max(0, -dx), 7 - max(0, dx))
                        ny, nx = ys.stop - ys.start, xs.stop - xs.start
                        o0 = 7 * ys.start + xs.start
                        rhs = bass.AP(
                            tensor=srcb.tensor,
                            offset=(srcb.offset + PADG + 8 * (ys.start + dy)
                                    + xs.start + dx),
                            ap=[[srcb.ap[0][0], 128], [SL, 8], [8, ny],
                                [1, nx]])
                        out_ap = bass.AP(
                            tensor=op2.tensor, offset=op2.offset + o0,
                            ap=[[op2.ap[0][0], 128], [N, 8], [7, ny],
                                [1, nx]])
                        nc.tensor.matmul(
                            out_ap,
                            dw_t[:, ct * len(TAPS_PE) + ti, :],
                            rhs, start=(ti == 0), stop=False)
                ck = chk[jo][hh][sl]
                for pr in range(4):
                    p = 4 * g + pr
                    rhs = bass.AP(tensor=ck.tensor,
                                  offset=ck.offset + 49 * pr,
                                  ap=[[ck.ap[0][0], 128], [4 * N, 2], [1, N]])
                    nc.tensor.matmul(
                        op2[:, 98 * pr: 98 * pr + 98],
                        vtp[p % 8][:, ct * 128:(ct + 1) * 128],
                        rhs, start=(stage < 8), stop=True)
                accv = acc_t[ct].rearrange("p (i q) -> p i q", q=SL)
                accv = accv.rearrange("p i (y x) -> p i y x",
                                      x=8)[:, :, 0:7, 0:7]
                tmp = mid.tile([128, GW], bf16, name="tmp", tag="tmp", bufs=5)
                nc.vector.tensor_add(cview(tmp),
                                     op2[:, 0:GW].rearrange(
                                         "p (i y x) -> p i y x", y=7, x=7),
                                     accv)
                rl = mid.tile([128, GW], bf16, name="rl", tag="rl", bufs=10)
                nc.vector.tensor_scalar(out=rl, in0=tmp,
                                        scalar1=crl_t[:, ct:ct + 1],
                                        scalar2=0.0,
                                        op0=ALU.add, op1=ALU.max)
                relu_t.append(rl)

            # --- projection + store ---
            for mt in range(3):
                st = stg.tile([128, GW], bf16, name="st", tag="st")
                pp_ = ps.tile([128, 512], f32, name="ps", tag="ps")
                for kt in range(8):
                    nc.tensor.matmul(pp_[:, 0:GW],
                                     wp_t[kt][:, mt * 128:(mt + 1) * 128],
                                     relu_t[kt],
                                     start=(kt == 0), stop=(kt == 7))
                nc.scalar.activation(st, pp_[:, 0:GW], AF.Identity,
                                     bias=bp_t[:, mt:mt + 1])
                dma(out=out_d[mt][:, c0:c0 + GW], in_=st)

        if loop_n > 1:
            with tc.For_i(0, loop_n, 1):
                for g in range(NG):
                    group_body(g)
        else:
            with tc.high_priority(200):
                group_body(0)
            for g in range(1, NG):
                group_body(g)

    nc.compile()
    return nc


_CACHE = {}


def _get_program(n_imgs):
    if n_imgs not in _CACHE:
        _CACHE[n_imgs] = build_program(n_imgs)
    return _CACHE[n_imgs]


_CONSTS_CACHE = {}


def _cached_consts(inputs):
    w = np.asarray(inputs['q_w'])
    key = (w.shape, w.dtype.str, w.tobytes()[:256])
    if key not in _CONSTS_CACHE:
        _CONSTS_CACHE.clear()
        _CONSTS_CACHE[key] = make_consts(inputs)
    return _CONSTS_CACHE[key]


def make_in_maps(inputs, n_cores=NCORES):
    """Host prep: shard + channel-major x, build replicated constants."""
    consts = _cached_consts(inputs)
    x = np.asarray(inputs['x'], np.float32)
    B = x.shape[0]
    ni = B // n_cores
    nt = ni * N
    x = x.reshape(B, N, DIM)
    in_maps = []
    for c in range(n_cores):
        m = dict(consts)
        xc = x[c * ni:(c + 1) * ni].reshape(nt, DIM).T    # [384, nt]
        m['x'] = np.ascontiguousarray(xc).reshape(3, 128, nt).astype(_BF16)
        in_maps.append(m)
    return in_maps, ni


def assemble_out(results, ni):
    """[3,128,nt] bf16 per core -> full [B, R, R, DIM] f32.

    Per-group image order on device is IPERM (pair-major for the
    checkerboard attn*V); un-permute here.
    """
    nt = ni * N
    ng = ni // 8
    outs = []
    for r in results:
        oc = np.asarray(r['out'], np.float32).reshape(DIM, nt)
        o = oc.T.reshape(ng, 8, N, DIM)[:, IPERM_INV]
        outs.append(o.reshape(ni, R, R, DIM))
    return np.concatenate(outs, axis=0)


def kernel(**inputs):
    from concourse import bass_utils
    in_maps, ni = make_in_maps(inputs)
    nc = _get_program(ni)
    res = bass_utils.run_bass_kernel_spmd(
        nc, in_maps, core_ids=list(range(NCORES)))
    return assemble_out(res.results, ni).astype(np.float32)

